# revision 30
# baseline (speedup 1.0000x reference)
"""Multi-head attention (b=4, h=8, d=64, n=2048, dim=256) on 8 TRN2 NeuronCores.

Sharding: core c handles batch b=c//2 and head-group g=c%2 (4 heads).
Each core computes its 4 heads' attention plus the partial output
projection y_part = w_out[:, g*256:(g+1)*256] @ attn_out, returned
transposed as yT [n, 256].  Host: y[b] = (yT[2b] + yT[2b+1]).T + b_out.
No cross-core collectives (the hint's b*h split, done host-side).
All inputs are pre-cast to bf16 on the host during sharding.

Per-core pipeline (n=2048, 4 heads as 2 head-pairs, matmul operands bf16,
PSUM accumulation fp32).  The AV stage exploits the PE cost law
(cycles = moving/output free size, stationary loads are free):
  QKV:    q,k via w-stationary matmuls; v^T via x-stationary matmuls
          (v lands in the [j, d] layout AV needs).  v^T stored with a
          ones column per head ([128, 4*65]) so AV also computes Z free.
  scores: sim_T[j, i] = k^T q, two heads of a pair in one [128, 1024]
          PSUM supertile (512 i-cols each).
  exp:    one instruction per supertile -> bf16 SBUF.  Max-subtraction
          skipped (|sim| <~ 8).  Tiles are split across THREE engines:
          ACT runs true Exp; DVE and GPSIMD(Pool) run a one-instruction
          Schraudolph exp (int16 bit-trick -> bf16).  Scattered approx
          tiles dilute the ~1.7% elementwise error through the softmax
          denominator.
  AV:     TRANSPOSED layout: av[i, d|Z] += ex_tile^T @ vT with the
          128-col ex slice stationary and vT [128, 65] moving -> 65
          cycles per matmul instead of 512 (8 small matmuls per
          supertile, PSUM-accumulated over the 16 j-tiles; i on
          partitions).  This halves+ the AV PE time vs the classic
          orientation.
  norm:   Z sits in PSUM col 64 of each av slot; one strided DVE
          reciprocal -> invZ [128, 8] SBUF, then 8 per-partition-scaled
          copies (GPSIMD tensor_scalar ptr) write normalized bf16 into
          packed [128 i, 128 c] tiles.
  trans:  PE transpose (identity matmul, 128 cycles) flips each packed
          tile to [c, i] into PSUM-bf16; DVE evicts (2x mode) into the
          persistent out_cloc [128 c, 2048 i] per pair.
  proj:   yT[i, o] with out_cloc stationary, w_outT moving; f32 evict,
          DMA per i-tile.

Scheduling: QKV is deferred piecewise into the first loops' exp-wait
windows; AV runs a 3-iteration software pipeline behind exp and persists
across loop boundaries; norm of loop L runs at loop L+1 jt=3 (before
L+1's first AV write reuses the single av PSUM buffer), transposes at
jt=4/5, projection of chunk ic at the next p==0 loop jts 12..15.
PSUM: 3x2-bank sim supertiles (also borrowed by deferred QKV, transpose
and projection pieces) + 2 av banks = 8 banks exactly.
"""

import numpy as np
from contextlib import ExitStack

DIM = 256
HEADS = 8
DH = 64
HID = 512
N = 2048
B = 4
SCALE = DH ** -0.5
P = 128
NI = N // 512   # 4 i-chunks of 512
NJ = N // P     # 16 j-tiles of 128

_CACHE = {}


def _build_nc(repeat=1):
    if repeat in _CACHE:
        return _CACHE[repeat]
    import concourse.tile as tile
    from concourse import bacc, mybir
    from concourse.masks import make_identity

    f32 = mybir.dt.float32
    bf16 = mybir.dt.bfloat16
    i16 = mybir.dt.int16
    Exp = mybir.ActivationFunctionType.Exp
    # Schraudolph exp approximation constants (bf16 bit pattern via int16):
    # i16 = round(x * 2^7/ln2 + (127*2^7 - C)); bits reinterpreted as bf16.
    SCH_A = float(2 ** 7 / np.log(2))
    SCH_B = float(127 * 2 ** 7 - 486408.0 / 65536.0)
    # exp engine per j-tile in steady loops (first loop runs all-ACT).
    # GPSIMD cannot touch PSUM on TRN2, so only ACT (true exp) and DVE
    # (Schraudolph) share the exp work.
    SCH_DVE_JTS = (0, 2, 4, 6, 8, 10, 13)
    SCH_DVE_JTS_FIRST = (5, 9, 13)

    nc = bacc.Bacc("TRN2", target_bir_lowering=False, debug=False)
    # hd packs [wqkT | x chunk0] so the first-matmul dependency set arrives
    # in one DMA per row-tile (HWDGE serializes at ~625ns/DMA).
    hd_d = nc.dram_tensor("hd", [DIM, 1024], bf16, kind="ExternalInput").ap()
    xc1_d = nc.dram_tensor("xc1", [P, 1024], bf16, kind="ExternalInput").ap()
    wv_d = nc.dram_tensor("wv2", [P, 512], bf16, kind="ExternalInput").ap()
    xr_d = nc.dram_tensor("xrest", [P, 2560], bf16, kind="ExternalInput").ap()
    y_d = nc.dram_tensor("yT", [N, DIM], bf16, kind="ExternalOutput").ap()

    with tile.TileContext(nc) as tc, ExitStack() as ctx:
        persist = ctx.enter_context(tc.tile_pool(name="persist", bufs=1))

        # dependency-free warm-up first: ramp the PE p-state through the
        # DMA-wait window (memset on the otherwise-idle Pool engine)
        warm = persist.tile([P, P], bf16, tag="warm", name="warm")
        nc.gpsimd.memset(warm[:], 0.0)

        # head DMAs: 4 total ([wqkT|x_c0|wvT] and [x_c123|woutT] per
        # row-tile), everything lands directly as bf16, no on-chip casts
        wqk = []
        xsb = {}
        wv = []
        wob = []
        for r in range(2):
            hb = persist.tile([P, 1024], bf16, tag=f"hdb{r}", name=f"hdb{r}")
            nc.sync.dma_start(hb[:], hd_d[r * P:(r + 1) * P, :])
            wqk.append(hb[:, 0:512])
            xsb[(r, 0)] = hb[:, 512:1024]
        xc1 = persist.tile([P, 1024], bf16, tag="xc1", name="xc1")
        nc.sync.dma_start(xc1[:], xc1_d[:, :])
        wvt = persist.tile([P, 512], bf16, tag="wv2", name="wv2")
        nc.sync.dma_start(wvt[:], wv_d[:, :])
        wv.append(wvt[:, 0:256])
        wv.append(wvt[:, 256:512])
        xrest = persist.tile([P, 2560], bf16, tag="xrest", name="xrest")
        nc.sync.dma_start(xrest[:], xr_d[:, :])
        for r in range(2):
            xsb[(r, 1)] = xc1[:, r * 512:(r + 1) * 512]
            for c in range(2, NI):
                xsb[(r, c)] = xrest[:, (c - 2) * 1024 + r * 512:(c - 2) * 1024 + (r + 1) * 512]
            wob.append(xrest[:, 2048 + r * 256:2048 + (r + 1) * 256])

        # identity for the tail's PE transposes (built during the DMA wait)
        ident = persist.tile([P, P], bf16, tag="ident", name="ident")
        make_identity(nc, ident[:])

        # ---- Stage A: minimal upfront QKV; the rest is interleaved into the
        # early attention loops' exp-wait windows.
        qkt = {}
        vT = [None] * NJ

        def emit_qk(pool, m, c, evict=None):
            ps = pool.tile([P, 512], f32, tag=pool._qkv_tag, name="qkps")
            for r in range(2):
                nc.tensor.matmul(
                    ps[:],
                    wqk[r][:, m * P:(m + 1) * P],
                    xsb[(r, c)],
                    start=(r == 0), stop=(r == 1),
                )
            t = persist.tile([P, 512], bf16, tag=f"qk{m}_{c}", name=f"qk{m}_{c}")
            if evict is nc.scalar:
                nc.scalar.copy(t[:], ps[:])
            else:
                (evict or nc.vector).tensor_copy(t[:], ps[:])
            qkt[(m, c)] = t

        def emit_v(pool, j, evict=None):
            ps = pool.tile([P, 256], f32, tag=pool._qkv_tag, name="vps")
            for r in range(2):
                nc.tensor.matmul(
                    ps[:],
                    xsb[(r, j // 4)][:, (j % 4) * P:(j % 4 + 1) * P],
                    wv[r][:],
                    start=(r == 0), stop=(r == 1),
                )
            t = persist.tile([P, 4 * 65], bf16, tag=f"vT{j}", name=f"vT{j}")
            tv = t[:].rearrange("p (h w) -> p h w", h=4)
            nc.gpsimd.memset(tv[:, :, 64:65], 1.0)
            (evict or nc.vector).tensor_copy(tv[:, :, 0:64], ps[:].rearrange("p (h w) -> p h w", h=4))
            vT[j] = t

        with tc.tile_pool(name="qkvps", bufs=2, space="PSUM") as qp:
            qp._qkv_tag = "qkps"
            # dependency-free warm-up matmuls: ramp the PE p-state past the
            # ~3us HAM window before the first real (DMA-gated) matmul lands
            wps = qp.tile([P, 512], f32, tag="qkps", name="warmps")
            for wi in range(26):
                nc.tensor.matmul(wps[:, 0:P], warm[:], warm[:],
                                 start=(wi == 0), stop=(wi == 25))
            emit_qk(qp, 2, 0, evict=nc.scalar)   # k heads 0,1 chunk 0
            emit_qk(qp, 0, 0)                    # q heads 0,1 chunk 0

        # ---- Stage B: attention + projection ----
        on = []
        for p2 in range(2):
            t = persist.tile([P, N], bf16, tag=f"on{p2}", name=f"on{p2}")
            on.append(t)
        small = ctx.enter_context(tc.tile_pool(name="small", bufs=4))
        tinp = ctx.enter_context(tc.tile_pool(name="tinp", bufs=3))
        expool = ctx.enter_context(tc.tile_pool(name="expool", bufs=24))
        simp = ctx.enter_context(tc.tile_pool(name="simp", bufs=3, space="PSUM"))
        avp = ctx.enter_context(tc.tile_pool(name="avp", bufs=1, space="PSUM"))
        yout = ctx.enter_context(tc.tile_pool(name="yout", bufs=3))

        yp = simp  # aux PSUM (deferred QKV, transpose, proj) borrows sim slots
        yp._qkv_tag = "sim"
        # Deferred QKV pieces, interleaved into the early attention loops
        # between exp(jt) and av(jt) where the PE waits on exp anyway.
        # Constraints: v(j) before AV(j) at jt=j+3; k01_cX before jt=4X;
        # k-tiles of a pair before that pair's loop; q_cX before ic=X.
        deferred = {
            (0, 0): {
                0: [("v", 0)],
                1: [("v", 1), ("v", 2)],
                2: [("v", 3), ("qk", 2, 1)],
                3: [("v", 4), ("v", 5)],
                4: [("v", 6), ("qk", 2, 2)],
                5: [("v", 7), ("v", 8)],
                6: [("v", 9), ("qk", 2, 3)],
                7: [("v", 10), ("v", 11)],
                8: [("v", 12), ("qk", 3, 0)],
                9: [("v", 13), ("v", 14)],
                10: [("v", 15), ("qk", 3, 1)],
                11: [("qk", 3, 2)],
                12: [("qk", 3, 3)],
                13: [("qk", 1, 0)],
                14: [("qk", 0, 1)],
            },
            (0, 1): {1: [("qk", 1, 1)]},   # q23_c1, used at (1,1)
            (1, 0): {1: [("qk", 0, 2)]},   # q01_c2, used at (2,0)
            (1, 1): {1: [("qk", 1, 2)]},   # q23_c2, used at (2,1)
            (2, 0): {1: [("qk", 0, 3)]},   # q01_c3, used at (3,0)
            (2, 1): {1: [("qk", 1, 3)]},   # q23_c3, used at (3,1)
        }

        pend_ydma = []
        ysb_cur = [None]

        def emit_proj(ic, s, act_evict=False):
            i0 = ic * 512 + s * P
            pyp = yp.tile([P, 256], f32, tag="sim", name="yproj")
            for ct in range(2):
                nc.tensor.matmul(pyp[:], on[ct][:, i0:i0 + P], wob[ct][:],
                                 start=(ct == 0), stop=(ct == 1))
            if s == 0:
                ysb_cur[0] = yout.tile([P, 1024], bf16, tag="ysb", name="ysb")
            ysb = ysb_cur[0]
            with tc.high_priority():
                # jump the engine queue: the sim-ring slot pyp borrows stays
                # blocked until this evict runs
                if act_evict:
                    nc.scalar.copy(ysb[:, s * 256:(s + 1) * 256], pyp[:])
                else:
                    nc.vector.tensor_copy(ysb[:, s * 256:(s + 1) * 256], pyp[:])
            if s == 3:
                # one DMA for the whole 512-row chunk: y rows i = s*128 + p
                pend_ydma.append((ic, ysb))

        def emit_ydma():
            ic, ysb = pend_ydma.pop(0)
            nc.sync.dma_start(
                y_d[ic * 512:(ic + 1) * 512, :].rearrange("(s p) o -> p s o", s=4),
                ysb[:].rearrange("p (s o) -> p s o", s=4))

        def emit_av(entry):
            # transposed AV: av[i, d|Z] += ex_slice^T @ vT_head (65 cycles).
            # PSUM lazy-zeroing is per 2KB bank: only the first matmul of a
            # bank may set start, only the last sets stop.
            ex, jt2, av, pp = entry
            for h2 in range(2):
                hh = 2 * pp + h2
                for s in range(4):
                    off = h2 * 512 + s * 128
                    nc.tensor.matmul(
                        av[:, off:off + 65],
                        ex[:, off:off + 128],
                        vT[jt2][:, hh * 65:hh * 65 + 65],
                        start=(jt2 == 0 and s == 0),
                        stop=(jt2 == NJ - 1 and s == 3),
                    )

        def emit_norm(entry, split4=False):
            # invZ for all 8 slots in one strided reciprocal, then
            # broadcast-multiplies into the packed [i, c] tile.
            pp, icc, av = entry
            avv = av[:].rearrange("q (h s c) -> q h s c", h=2, s=4)
            iz = small.tile([P, 8], f32, tag="iz", name="iz")
            nc.vector.reciprocal(
                iz[:].rearrange("q (h s c) -> q h s c", h=2, s=4),
                avv[:, :, :, 64:65])
            # one packed [i, 4s x (2h x 64c)] tile; 2 broadcast-multiply ops
            # (one per h2) replace 8 per-slot scaled copies
            ta = tinp.tile([P, 512], bf16, tag="tins", name="tins")
            tav = ta[:].rearrange("q (s t c) -> q s t c", s=4, t=2)
            # dim order (s, h2, c) on BOTH sides -> a single strided
            # TensorTensor normalizes all 8 slots (no inter-op queue gap
            # on the loop-boundary critical path)
            avp_ = av[:].rearrange("q (h s c) -> q s h c", h=2, s=4)
            izp = iz[:].rearrange("q (h s) -> q s h ()", h=2)
            if not split4:
                nc.vector.tensor_tensor(
                    tav[:, :, :, :],
                    avp_[:, :, :, 0:64],
                    izp.broadcast_to([P, 4, 2, 64]),
                    mybir.AluOpType.mult)
            else:
                # tail: two s-pair pieces so the first transposes start early
                for s0, s1 in ((0, 2), (2, 4)):
                    ns = s1 - s0
                    nc.vector.tensor_tensor(
                        tav[:, s0:s1, :, :],
                        avp_[:, s0:s1, :, 0:64],
                        izp[:, s0:s1].broadcast_to([P, ns, 2, 64]),
                        mybir.AluOpType.mult)
            return [(pp, icc, s, ta) for s in range(4)]

        def emit_trans(job):
            # [i, c] -> [c, i] via the DMA XBAR (SBUF->SBUF, bf16): zero
            # engine cost, only HWDGE ring occupancy.
            pp, icc, s, ta = job
            nc.sync.dma_start_transpose(
                out=on[pp][:, icc * 512 + s * P: icc * 512 + (s + 1) * P],
                in_=ta[:, s * P:(s + 1) * P])

        # Software pipelines persisting ACROSS (ic, p) loops: av matmuls run
        # 3 jts behind their exp; the norm of loop L runs at loop L+1 jt=3
        # (just after L's last AV, emitted at L+1 jt=2, and before L+1's
        # AV(0) reclaims the single av PSUM buffer); transposes at jt=4/5.
        pend_av = []
        pend_norm = []
        pend_trans = []
        pending_proj = None
        for rep in range(repeat):
            for ic in range(NI):
                for p in range(2):
                    qt = qkt[(p, ic)]
                    av = avp.tile([P, 1024], f32, tag="av", name="av")
                    dmap = dict(deferred.get((ic, p), {})) if rep == 0 else {}
                    if p == 0 and pending_proj is not None:
                        for s in range(4):
                            dmap.setdefault(12 + s, []).append(("proj", pending_proj, s))
                        pending_proj = None
                    for jt in range(NJ):
                        sim = simp.tile([P, 1024], f32, tag="sim", name="sim")
                        kt = qkt[(2 + p, jt // 4)]
                        ko = (jt % 4) * P
                        nc.tensor.matmul(sim[:, 0:512], kt[0:64, ko:ko + P],
                                         qt[0:64, :], start=True, stop=True)
                        nc.tensor.matmul(sim[:, 512:1024], kt[64:128, ko:ko + P],
                                         qt[64:128, :], start=True, stop=True)
                        first_loop = (ic, p) == (0, 0) and rep == 0
                        sch_set = SCH_DVE_JTS_FIRST if first_loop else SCH_DVE_JTS
                        if jt in sch_set:
                            exi = expool.tile([P, 1024], i16, tag="ex", name="exi")
                            nc.vector.tensor_scalar(
                                exi[:], sim[:], SCH_A, SCH_B,
                                mybir.AluOpType.mult, mybir.AluOpType.add)
                            ex = exi[:].bitcast(bf16)
                        else:  # ACT true exp
                            exb = expool.tile([P, 1024], bf16, tag="ex", name="ex")
                            nc.scalar.activation(exb[:], sim[:], Exp)
                            ex = exb[:]
                        if jt == 0 and pend_ydma:
                            emit_ydma()
                        if jt in (5, 6) and pend_trans:
                            emit_trans(pend_trans.pop(0))
                            emit_trans(pend_trans.pop(0))
                        for piece in dmap.get(jt, []):
                            if piece[0] == "v":
                                emit_v(yp, piece[1])
                            elif piece[0] == "qk":
                                emit_qk(yp, piece[1], piece[2])
                            else:
                                emit_proj(piece[1], piece[2], act_evict=True)
                        pend_av.append((ex, jt, av, p))
                        if len(pend_av) > 4:
                            emit_av(pend_av.pop(0))
                    # drain the AV pipeline and normalize NOW (ahead of the
                    # next loop's exps in priority order) so the single av
                    # PSUM buffer frees early at the loop boundary.
                    while pend_av:
                        emit_av(pend_av.pop(0))
                    pend_trans.extend(emit_norm((p, ic, av),
                                                 split4=(ic == NI - 1 and p == 1)))
                    if p == 1:
                        pending_proj = ic
        # tail: low-latency PE transposes (borrowing sim PSUM slots), the
        # final projections interleaved per i-subtile, trans-evicts on ACT
        # (parallel to the DVE ysb evicts), and per-subtile output DMAs so
        # the last DMA is small and earlier ones overlap the tail compute
        for s in range(4):
            pp, icc, s_, ta = pend_trans.pop(0)
            tp = yp.tile([P, P], bf16, tag="sim", name="tps")
            nc.tensor.transpose(tp[:], ta[:, s_ * P:(s_ + 1) * P], ident[:])
            nc.vector.tensor_copy(
                on[pp][:, icc * 512 + s_ * P: icc * 512 + (s_ + 1) * P], tp[:])
            emit_proj(pending_proj, s_, act_evict=True)
        while pend_ydma:
            pend_ydma.pop(0)
        ic3, ysb = pending_proj, ysb_cur[0]
        nc.sync.dma_start(
            y_d[ic3 * 512:ic3 * 512 + 256, :].rearrange("(s p) o -> p s o", s=2),
            ysb[:, 0:512].rearrange("p (s o) -> p s o", s=2))
        for s_ in (2, 3):
            r0 = ic3 * 512 + s_ * P
            nc.sync.dma_start(y_d[r0:r0 + P, :], ysb[:, s_ * 256:(s_ + 1) * 256])

    nc.compile()
    _CACHE[repeat] = nc
    return nc


def _shard_inputs(x, w_qkv, w_out):
    import ml_dtypes
    bf16 = ml_dtypes.bfloat16
    in_maps = []
    for c in range(8):
        b, g = c // 2, c % 2
        wq = w_qkv[g * 256:(g + 1) * 256] * SCALE
        wk = w_qkv[512 + g * 256:512 + (g + 1) * 256]
        wvv = w_qkv[1024 + g * 256:1024 + (g + 1) * 256]
        wqkT = np.concatenate([wq, wk], 0).T
        hd = np.concatenate([wqkT, x[b][:, 0:512]], axis=1)
        xb = x[b]
        woT = w_out[:, g * 256:(g + 1) * 256].T
        xc1 = np.concatenate([xb[0:128, 512:1024], xb[128:256, 512:1024]], axis=1)
        xrest = np.concatenate(
            [xb[0:128, 1024:1536], xb[128:256, 1024:1536],
             xb[0:128, 1536:2048], xb[128:256, 1536:2048],
             woT[0:128], woT[128:256]], axis=1)
        wvT = wvv.T
        in_maps.append({
            "hd": np.ascontiguousarray(hd.astype(bf16)),
            "xc1": np.ascontiguousarray(xc1.astype(bf16)),
            "wv2": np.ascontiguousarray(
                np.concatenate([wvT[0:128], wvT[128:256]], axis=1).astype(bf16)),
            "xrest": np.ascontiguousarray(xrest.astype(bf16)),
        })
    return in_maps


def kernel(x, w_qkv, w_out, b_out):
    from concourse.bass_utils import run_bass_kernel_spmd
    x = np.asarray(x, dtype=np.float32)
    w_qkv = np.asarray(w_qkv, dtype=np.float32)
    w_out = np.asarray(w_out, dtype=np.float32)
    b_out = np.asarray(b_out, dtype=np.float32)

    nc = _build_nc()
    in_maps = _shard_inputs(x, w_qkv, w_out)
    res = run_bass_kernel_spmd(nc, in_maps, core_ids=list(range(8)))
    y = np.empty((B, DIM, N), np.float32)
    for b in range(B):
        yT = (np.asarray(res.results[2 * b]["yT"], dtype=np.float32)
              + np.asarray(res.results[2 * b + 1]["yT"], dtype=np.float32))
        y[b] = yT.T + b_out[:, None]
    return y


# revision 31
# speedup vs baseline: 1.0034x; 1.0034x over previous
"""Multi-head attention (b=4, h=8, d=64, n=2048, dim=256) on 8 TRN2 NeuronCores.

Sharding: core c handles batch b=c//2 and head-group g=c%2 (4 heads).
Each core computes its 4 heads' attention plus the partial output
projection y_part = w_out[:, g*256:(g+1)*256] @ attn_out, returned
transposed as yT [n, 256].  Host: y[b] = (yT[2b] + yT[2b+1]).T + b_out.
No cross-core collectives (the hint's b*h split, done host-side).
All inputs are pre-cast to bf16 on the host during sharding.

Per-core pipeline (n=2048, 4 heads as 2 head-pairs, matmul operands bf16,
PSUM accumulation fp32).  The AV stage exploits the PE cost law
(cycles = moving/output free size, stationary loads are free):
  QKV:    q,k via w-stationary matmuls; v^T via x-stationary matmuls
          (v lands in the [j, d] layout AV needs).  v^T stored with a
          ones column per head ([128, 4*65]) so AV also computes Z free.
  scores: sim_T[j, i] = k^T q, two heads of a pair in one [128, 1024]
          PSUM supertile (512 i-cols each).
  exp:    one instruction per supertile -> bf16 SBUF.  Max-subtraction
          skipped (|sim| <~ 8).  Tiles are split across THREE engines:
          ACT runs true Exp; DVE and GPSIMD(Pool) run a one-instruction
          Schraudolph exp (int16 bit-trick -> bf16).  Scattered approx
          tiles dilute the ~1.7% elementwise error through the softmax
          denominator.
  AV:     TRANSPOSED layout: av[i, d|Z] += ex_tile^T @ vT with the
          128-col ex slice stationary and vT [128, 65] moving -> 65
          cycles per matmul instead of 512 (8 small matmuls per
          supertile, PSUM-accumulated over the 16 j-tiles; i on
          partitions).  This halves+ the AV PE time vs the classic
          orientation.
  norm:   Z sits in PSUM col 64 of each av slot; one strided DVE
          reciprocal -> invZ [128, 8] SBUF, then 8 per-partition-scaled
          copies (GPSIMD tensor_scalar ptr) write normalized bf16 into
          packed [128 i, 128 c] tiles.
  trans:  PE transpose (identity matmul, 128 cycles) flips each packed
          tile to [c, i] into PSUM-bf16; DVE evicts (2x mode) into the
          persistent out_cloc [128 c, 2048 i] per pair.
  proj:   yT[i, o] with out_cloc stationary, w_outT moving; f32 evict,
          DMA per i-tile.

Scheduling: QKV is deferred piecewise into the first loops' exp-wait
windows; AV runs a 3-iteration software pipeline behind exp and persists
across loop boundaries; norm of loop L runs at loop L+1 jt=3 (before
L+1's first AV write reuses the single av PSUM buffer), transposes at
jt=4/5, projection of chunk ic at the next p==0 loop jts 12..15.
PSUM: 3x2-bank sim supertiles (also borrowed by deferred QKV, transpose
and projection pieces) + 2 av banks = 8 banks exactly.
"""

import numpy as np
from contextlib import ExitStack

DIM = 256
HEADS = 8
DH = 64
HID = 512
N = 2048
B = 4
SCALE = DH ** -0.5
P = 128
NI = N // 512   # 4 i-chunks of 512
NJ = N // P     # 16 j-tiles of 128

_CACHE = {}


def _build_nc(repeat=1):
    if repeat in _CACHE:
        return _CACHE[repeat]
    import concourse.tile as tile
    from concourse import bacc, mybir
    from concourse.masks import make_identity

    f32 = mybir.dt.float32
    bf16 = mybir.dt.bfloat16
    i16 = mybir.dt.int16
    Exp = mybir.ActivationFunctionType.Exp
    # Schraudolph exp approximation constants (bf16 bit pattern via int16):
    # i16 = round(x * 2^7/ln2 + (127*2^7 - C)); bits reinterpreted as bf16.
    SCH_A = float(2 ** 7 / np.log(2))
    SCH_B = float(127 * 2 ** 7 - 486408.0 / 65536.0)
    # exp engine per j-tile in steady loops (first loop runs all-ACT).
    # GPSIMD cannot touch PSUM on TRN2, so only ACT (true exp) and DVE
    # (Schraudolph) share the exp work.
    SCH_DVE_JTS = (1, 3, 5, 7, 9, 11, 14)
    SCH_DVE_JTS_FIRST = (5, 9, 13)

    nc = bacc.Bacc("TRN2", target_bir_lowering=False, debug=False)
    # hd packs [wqkT | x chunk0] so the first-matmul dependency set arrives
    # in one DMA per row-tile (HWDGE serializes at ~625ns/DMA).
    hd_d = nc.dram_tensor("hd", [DIM, 1024], bf16, kind="ExternalInput").ap()
    xc1_d = nc.dram_tensor("xc1", [P, 1024], bf16, kind="ExternalInput").ap()
    wv_d = nc.dram_tensor("wv2", [P, 512], bf16, kind="ExternalInput").ap()
    xr_d = nc.dram_tensor("xrest", [P, 2560], bf16, kind="ExternalInput").ap()
    y_d = nc.dram_tensor("yT", [N, DIM], bf16, kind="ExternalOutput").ap()

    with tile.TileContext(nc) as tc, ExitStack() as ctx:
        persist = ctx.enter_context(tc.tile_pool(name="persist", bufs=1))

        # dependency-free warm-up first: ramp the PE p-state through the
        # DMA-wait window (memset on the otherwise-idle Pool engine)
        warm = persist.tile([P, P], bf16, tag="warm", name="warm")
        nc.gpsimd.memset(warm[:], 0.0)

        # head DMAs: 4 total ([wqkT|x_c0|wvT] and [x_c123|woutT] per
        # row-tile), everything lands directly as bf16, no on-chip casts
        wqk = []
        xsb = {}
        wv = []
        wob = []
        for r in range(2):
            hb = persist.tile([P, 1024], bf16, tag=f"hdb{r}", name=f"hdb{r}")
            nc.sync.dma_start(hb[:], hd_d[r * P:(r + 1) * P, :])
            wqk.append(hb[:, 0:512])
            xsb[(r, 0)] = hb[:, 512:1024]
        xc1 = persist.tile([P, 1024], bf16, tag="xc1", name="xc1")
        nc.sync.dma_start(xc1[:], xc1_d[:, :])
        wvt = persist.tile([P, 512], bf16, tag="wv2", name="wv2")
        nc.sync.dma_start(wvt[:], wv_d[:, :])
        wv.append(wvt[:, 0:256])
        wv.append(wvt[:, 256:512])
        xrest = persist.tile([P, 2560], bf16, tag="xrest", name="xrest")
        nc.sync.dma_start(xrest[:], xr_d[:, :])
        for r in range(2):
            xsb[(r, 1)] = xc1[:, r * 512:(r + 1) * 512]
            for c in range(2, NI):
                xsb[(r, c)] = xrest[:, (c - 2) * 1024 + r * 512:(c - 2) * 1024 + (r + 1) * 512]
            wob.append(xrest[:, 2048 + r * 256:2048 + (r + 1) * 256])

        # identity for the tail's PE transposes (built during the DMA wait)
        ident = persist.tile([P, P], bf16, tag="ident", name="ident")
        make_identity(nc, ident[:])

        # ---- Stage A: minimal upfront QKV; the rest is interleaved into the
        # early attention loops' exp-wait windows.
        qkt = {}
        vT = [None] * NJ

        def emit_qk(pool, m, c, evict=None):
            ps = pool.tile([P, 512], f32, tag=pool._qkv_tag, name="qkps")
            for r in range(2):
                nc.tensor.matmul(
                    ps[:],
                    wqk[r][:, m * P:(m + 1) * P],
                    xsb[(r, c)],
                    start=(r == 0), stop=(r == 1),
                )
            t = persist.tile([P, 512], bf16, tag=f"qk{m}_{c}", name=f"qk{m}_{c}")
            if evict is nc.scalar:
                nc.scalar.copy(t[:], ps[:])
            else:
                (evict or nc.vector).tensor_copy(t[:], ps[:])
            qkt[(m, c)] = t

        def emit_v(pool, j, evict=None):
            ps = pool.tile([P, 256], f32, tag=pool._qkv_tag, name="vps")
            for r in range(2):
                nc.tensor.matmul(
                    ps[:],
                    xsb[(r, j // 4)][:, (j % 4) * P:(j % 4 + 1) * P],
                    wv[r][:],
                    start=(r == 0), stop=(r == 1),
                )
            t = persist.tile([P, 4 * 65], bf16, tag=f"vT{j}", name=f"vT{j}")
            tv = t[:].rearrange("p (h w) -> p h w", h=4)
            nc.gpsimd.memset(tv[:, :, 64:65], 1.0)
            (evict or nc.vector).tensor_copy(tv[:, :, 0:64], ps[:].rearrange("p (h w) -> p h w", h=4))
            vT[j] = t

        with tc.tile_pool(name="qkvps", bufs=2, space="PSUM") as qp:
            qp._qkv_tag = "qkps"
            # dependency-free warm-up matmuls: ramp the PE p-state past the
            # ~3us HAM window before the first real (DMA-gated) matmul lands
            wps = qp.tile([P, 512], f32, tag="qkps", name="warmps")
            for wi in range(26):
                nc.tensor.matmul(wps[:, 0:P], warm[:], warm[:],
                                 start=(wi == 0), stop=(wi == 25))
            emit_qk(qp, 2, 0, evict=nc.scalar)   # k heads 0,1 chunk 0
            emit_qk(qp, 0, 0)                    # q heads 0,1 chunk 0

        # ---- Stage B: attention + projection ----
        on = []
        for p2 in range(2):
            t = persist.tile([P, N], bf16, tag=f"on{p2}", name=f"on{p2}")
            on.append(t)
        small = ctx.enter_context(tc.tile_pool(name="small", bufs=4))
        tinp = ctx.enter_context(tc.tile_pool(name="tinp", bufs=3))
        expool = ctx.enter_context(tc.tile_pool(name="expool", bufs=16))
        simp = ctx.enter_context(tc.tile_pool(name="simp", bufs=3, space="PSUM"))
        avp = ctx.enter_context(tc.tile_pool(name="avp", bufs=1, space="PSUM"))
        yout = ctx.enter_context(tc.tile_pool(name="yout", bufs=3))

        yp = simp  # aux PSUM (deferred QKV, transpose, proj) borrows sim slots
        yp._qkv_tag = "sim"
        # Deferred QKV pieces, interleaved into the early attention loops
        # between exp(jt) and av(jt) where the PE waits on exp anyway.
        # Constraints: v(j) before AV(j) at jt=j+3; k01_cX before jt=4X;
        # k-tiles of a pair before that pair's loop; q_cX before ic=X.
        deferred = {
            (0, 0): {
                0: [("v", 0)],
                1: [("v", 1), ("v", 2)],
                2: [("v", 3), ("qk", 2, 1)],
                3: [("v", 4), ("v", 5)],
                4: [("v", 6), ("qk", 2, 2)],
                5: [("v", 7), ("v", 8)],
                6: [("v", 9), ("qk", 2, 3)],
                7: [("v", 10), ("v", 11)],
                8: [("v", 12), ("qk", 3, 0)],
                9: [("v", 13), ("v", 14)],
                10: [("v", 15), ("qk", 3, 1)],
                11: [("qk", 3, 2)],
                12: [("qk", 3, 3)],
                13: [("qk", 1, 0)],
                14: [("qk", 0, 1)],
            },
            (0, 1): {1: [("qk", 1, 1)]},   # q23_c1, used at (1,1)
            (1, 0): {1: [("qk", 0, 2)]},   # q01_c2, used at (2,0)
            (1, 1): {1: [("qk", 1, 2)]},   # q23_c2, used at (2,1)
            (2, 0): {1: [("qk", 0, 3)]},   # q01_c3, used at (3,0)
            (2, 1): {1: [("qk", 1, 3)]},   # q23_c3, used at (3,1)
        }

        pend_ydma = []
        ysb_cur = [None]

        def emit_proj(ic, s, act_evict=False):
            i0 = ic * 512 + s * P
            pyp = yp.tile([P, 256], f32, tag="sim", name="yproj")
            for ct in range(2):
                nc.tensor.matmul(pyp[:], on[ct][:, i0:i0 + P], wob[ct][:],
                                 start=(ct == 0), stop=(ct == 1))
            if s == 0:
                ysb_cur[0] = yout.tile([P, 1024], bf16, tag="ysb", name="ysb")
            ysb = ysb_cur[0]
            with tc.high_priority():
                # jump the engine queue: the sim-ring slot pyp borrows stays
                # blocked until this evict runs
                if act_evict:
                    nc.scalar.copy(ysb[:, s * 256:(s + 1) * 256], pyp[:])
                else:
                    nc.vector.tensor_copy(ysb[:, s * 256:(s + 1) * 256], pyp[:])
            if s == 3:
                # one DMA for the whole 512-row chunk: y rows i = s*128 + p
                pend_ydma.append((ic, ysb))

        def emit_ydma():
            ic, ysb = pend_ydma.pop(0)
            nc.sync.dma_start(
                y_d[ic * 512:(ic + 1) * 512, :].rearrange("(s p) o -> p s o", s=4),
                ysb[:].rearrange("p (s o) -> p s o", s=4))

        def emit_av(entry):
            # transposed AV: av[i, d|Z] += ex_slice^T @ vT_head (65 cycles).
            # PSUM lazy-zeroing is per 2KB bank: only the first matmul of a
            # bank may set start, only the last sets stop.
            ex, jt2, av, pp = entry
            for h2 in range(2):
                hh = 2 * pp + h2
                for s in range(4):
                    off = h2 * 512 + s * 128
                    nc.tensor.matmul(
                        av[:, off:off + 65],
                        ex[:, off:off + 128],
                        vT[jt2][:, hh * 65:hh * 65 + 65],
                        start=(jt2 == 0 and s == 0),
                        stop=(jt2 == NJ - 1 and s == 3),
                    )

        def emit_norm(entry, split4=False):
            # invZ for all 8 slots in one strided reciprocal, then
            # broadcast-multiplies into the packed [i, c] tile.
            pp, icc, av = entry
            avv = av[:].rearrange("q (h s c) -> q h s c", h=2, s=4)
            iz = small.tile([P, 8], f32, tag="iz", name="iz")
            nc.vector.reciprocal(
                iz[:].rearrange("q (h s c) -> q h s c", h=2, s=4),
                avv[:, :, :, 64:65])
            # one packed [i, 4s x (2h x 64c)] tile; 2 broadcast-multiply ops
            # (one per h2) replace 8 per-slot scaled copies
            ta = tinp.tile([P, 512], bf16, tag="tins", name="tins")
            tav = ta[:].rearrange("q (s t c) -> q s t c", s=4, t=2)
            # dim order (s, h2, c) on BOTH sides -> a single strided
            # TensorTensor normalizes all 8 slots (no inter-op queue gap
            # on the loop-boundary critical path)
            avp_ = av[:].rearrange("q (h s c) -> q s h c", h=2, s=4)
            izp = iz[:].rearrange("q (h s) -> q s h ()", h=2)
            if not split4:
                nc.vector.tensor_tensor(
                    tav[:, :, :, :],
                    avp_[:, :, :, 0:64],
                    izp.broadcast_to([P, 4, 2, 64]),
                    mybir.AluOpType.mult)
            else:
                # tail: two s-pair pieces so the first transposes start early
                for s0, s1 in ((0, 2), (2, 4)):
                    ns = s1 - s0
                    nc.vector.tensor_tensor(
                        tav[:, s0:s1, :, :],
                        avp_[:, s0:s1, :, 0:64],
                        izp[:, s0:s1].broadcast_to([P, ns, 2, 64]),
                        mybir.AluOpType.mult)
            return [(pp, icc, s, ta) for s in range(4)]

        def emit_trans(job):
            # [i, c] -> [c, i] via the DMA XBAR (SBUF->SBUF, bf16): zero
            # engine cost, only HWDGE ring occupancy.
            pp, icc, s, ta = job
            nc.sync.dma_start_transpose(
                out=on[pp][:, icc * 512 + s * P: icc * 512 + (s + 1) * P],
                in_=ta[:, s * P:(s + 1) * P])

        # Software pipelines persisting ACROSS (ic, p) loops: av matmuls run
        # 3 jts behind their exp; the norm of loop L runs at loop L+1 jt=3
        # (just after L's last AV, emitted at L+1 jt=2, and before L+1's
        # AV(0) reclaims the single av PSUM buffer); transposes at jt=4/5.
        pend_av = []
        pend_norm = []
        pend_trans = []
        pending_proj = None
        for rep in range(repeat):
            for ic in range(NI):
                for p in range(2):
                    qt = qkt[(p, ic)]
                    av = avp.tile([P, 1024], f32, tag="av", name="av")
                    dmap = dict(deferred.get((ic, p), {})) if rep == 0 else {}
                    if p == 0 and pending_proj is not None:
                        for s in range(4):
                            dmap.setdefault(12 + s, []).append(("proj", pending_proj, s))
                        pending_proj = None
                    for jt in range(NJ):
                        sim = simp.tile([P, 1024], f32, tag="sim", name="sim")
                        kt = qkt[(2 + p, jt // 4)]
                        ko = (jt % 4) * P
                        nc.tensor.matmul(sim[:, 0:512], kt[0:64, ko:ko + P],
                                         qt[0:64, :], start=True, stop=True)
                        nc.tensor.matmul(sim[:, 512:1024], kt[64:128, ko:ko + P],
                                         qt[64:128, :], start=True, stop=True)
                        first_loop = (ic, p) == (0, 0) and rep == 0
                        sch_set = SCH_DVE_JTS_FIRST if first_loop else SCH_DVE_JTS
                        if jt in sch_set:
                            exi = expool.tile([P, 1024], i16, tag="ex", name="exi")
                            nc.vector.tensor_scalar(
                                exi[:], sim[:], SCH_A, SCH_B,
                                mybir.AluOpType.mult, mybir.AluOpType.add)
                            ex = exi[:].bitcast(bf16)
                        else:  # ACT true exp
                            exb = expool.tile([P, 1024], bf16, tag="ex", name="ex")
                            nc.scalar.activation(exb[:], sim[:], Exp)
                            ex = exb[:]
                        if jt == 0 and pend_ydma:
                            emit_ydma()
                        if jt in (7, 8) and pend_trans:
                            emit_trans(pend_trans.pop(0))
                            emit_trans(pend_trans.pop(0))
                        for piece in dmap.get(jt, []):
                            if piece[0] == "v":
                                emit_v(yp, piece[1])
                            elif piece[0] == "qk":
                                emit_qk(yp, piece[1], piece[2])
                            else:
                                emit_proj(piece[1], piece[2], act_evict=True)
                        pend_av.append((ex, jt, av, p))
                        if len(pend_av) > 4:
                            emit_av(pend_av.pop(0))
                    # drain the AV pipeline and normalize NOW (ahead of the
                    # next loop's exps in priority order) so the single av
                    # PSUM buffer frees early at the loop boundary.
                    while pend_av:
                        emit_av(pend_av.pop(0))
                    pend_trans.extend(emit_norm((p, ic, av),
                                                 split4=(ic == NI - 1 and p == 1)))
                    if p == 1:
                        pending_proj = ic
        # tail: low-latency PE transposes (borrowing sim PSUM slots), the
        # final projections interleaved per i-subtile, trans-evicts on ACT
        # (parallel to the DVE ysb evicts), and per-subtile output DMAs so
        # the last DMA is small and earlier ones overlap the tail compute
        for s in range(4):
            pp, icc, s_, ta = pend_trans.pop(0)
            tp = yp.tile([P, P], bf16, tag="sim", name="tps")
            nc.tensor.transpose(tp[:], ta[:, s_ * P:(s_ + 1) * P], ident[:])
            nc.vector.tensor_copy(
                on[pp][:, icc * 512 + s_ * P: icc * 512 + (s_ + 1) * P], tp[:])
            emit_proj(pending_proj, s_, act_evict=True)
        while pend_ydma:
            pend_ydma.pop(0)
        ic3, ysb = pending_proj, ysb_cur[0]
        nc.sync.dma_start(
            y_d[ic3 * 512:ic3 * 512 + 256, :].rearrange("(s p) o -> p s o", s=2),
            ysb[:, 0:512].rearrange("p (s o) -> p s o", s=2))
        for s_ in (2, 3):
            r0 = ic3 * 512 + s_ * P
            nc.sync.dma_start(y_d[r0:r0 + P, :], ysb[:, s_ * 256:(s_ + 1) * 256])

    nc.compile()
    _CACHE[repeat] = nc
    return nc


def _shard_inputs(x, w_qkv, w_out):
    import ml_dtypes
    bf16 = ml_dtypes.bfloat16
    in_maps = []
    for c in range(8):
        b, g = c // 2, c % 2
        wq = w_qkv[g * 256:(g + 1) * 256] * SCALE
        wk = w_qkv[512 + g * 256:512 + (g + 1) * 256]
        wvv = w_qkv[1024 + g * 256:1024 + (g + 1) * 256]
        wqkT = np.concatenate([wq, wk], 0).T
        hd = np.concatenate([wqkT, x[b][:, 0:512]], axis=1)
        xb = x[b]
        woT = w_out[:, g * 256:(g + 1) * 256].T
        xc1 = np.concatenate([xb[0:128, 512:1024], xb[128:256, 512:1024]], axis=1)
        xrest = np.concatenate(
            [xb[0:128, 1024:1536], xb[128:256, 1024:1536],
             xb[0:128, 1536:2048], xb[128:256, 1536:2048],
             woT[0:128], woT[128:256]], axis=1)
        wvT = wvv.T
        in_maps.append({
            "hd": np.ascontiguousarray(hd.astype(bf16)),
            "xc1": np.ascontiguousarray(xc1.astype(bf16)),
            "wv2": np.ascontiguousarray(
                np.concatenate([wvT[0:128], wvT[128:256]], axis=1).astype(bf16)),
            "xrest": np.ascontiguousarray(xrest.astype(bf16)),
        })
    return in_maps


def kernel(x, w_qkv, w_out, b_out):
    from concourse.bass_utils import run_bass_kernel_spmd
    x = np.asarray(x, dtype=np.float32)
    w_qkv = np.asarray(w_qkv, dtype=np.float32)
    w_out = np.asarray(w_out, dtype=np.float32)
    b_out = np.asarray(b_out, dtype=np.float32)

    nc = _build_nc()
    in_maps = _shard_inputs(x, w_qkv, w_out)
    res = run_bass_kernel_spmd(nc, in_maps, core_ids=list(range(8)))
    y = np.empty((B, DIM, N), np.float32)
    for b in range(B):
        yT = (np.asarray(res.results[2 * b]["yT"], dtype=np.float32)
              + np.asarray(res.results[2 * b + 1]["yT"], dtype=np.float32))
        y[b] = yT.T + b_out[:, None]
    return y


# revision 32
# speedup vs baseline: 1.0050x; 1.0016x over previous
"""Multi-head attention (b=4, h=8, d=64, n=2048, dim=256) on 8 TRN2 NeuronCores.

Sharding: core c handles batch b=c//2 and head-group g=c%2 (4 heads).
Each core computes its 4 heads' attention plus the partial output
projection y_part = w_out[:, g*256:(g+1)*256] @ attn_out, returned
transposed as yT [n, 256].  Host: y[b] = (yT[2b] + yT[2b+1]).T + b_out.
No cross-core collectives (the hint's b*h split, done host-side).
All inputs are pre-cast to bf16 on the host during sharding.

Per-core pipeline (n=2048, 4 heads as 2 head-pairs, matmul operands bf16,
PSUM accumulation fp32).  The AV stage exploits the PE cost law
(cycles = moving/output free size, stationary loads are free):
  QKV:    q,k via w-stationary matmuls; v^T via x-stationary matmuls
          (v lands in the [j, d] layout AV needs).  v^T stored with a
          ones column per head ([128, 4*65]) so AV also computes Z free.
  scores: sim_T[j, i] = k^T q, two heads of a pair in one [128, 1024]
          PSUM supertile (512 i-cols each).
  exp:    one instruction per supertile -> bf16 SBUF.  Max-subtraction
          skipped (|sim| <~ 8).  Tiles are split across THREE engines:
          ACT runs true Exp; DVE and GPSIMD(Pool) run a one-instruction
          Schraudolph exp (int16 bit-trick -> bf16).  Scattered approx
          tiles dilute the ~1.7% elementwise error through the softmax
          denominator.
  AV:     TRANSPOSED layout: av[i, d|Z] += ex_tile^T @ vT with the
          128-col ex slice stationary and vT [128, 65] moving -> 65
          cycles per matmul instead of 512 (8 small matmuls per
          supertile, PSUM-accumulated over the 16 j-tiles; i on
          partitions).  This halves+ the AV PE time vs the classic
          orientation.
  norm:   Z sits in PSUM col 64 of each av slot; one strided DVE
          reciprocal -> invZ [128, 8] SBUF, then 8 per-partition-scaled
          copies (GPSIMD tensor_scalar ptr) write normalized bf16 into
          packed [128 i, 128 c] tiles.
  trans:  PE transpose (identity matmul, 128 cycles) flips each packed
          tile to [c, i] into PSUM-bf16; DVE evicts (2x mode) into the
          persistent out_cloc [128 c, 2048 i] per pair.
  proj:   yT[i, o] with out_cloc stationary, w_outT moving; f32 evict,
          DMA per i-tile.

Scheduling: QKV is deferred piecewise into the first loops' exp-wait
windows; AV runs a 3-iteration software pipeline behind exp and persists
across loop boundaries; norm of loop L runs at loop L+1 jt=3 (before
L+1's first AV write reuses the single av PSUM buffer), transposes at
jt=4/5, projection of chunk ic at the next p==0 loop jts 12..15.
PSUM: 3x2-bank sim supertiles (also borrowed by deferred QKV, transpose
and projection pieces) + 2 av banks = 8 banks exactly.
"""

import numpy as np
from contextlib import ExitStack

DIM = 256
HEADS = 8
DH = 64
HID = 512
N = 2048
B = 4
SCALE = DH ** -0.5
P = 128
NI = N // 512   # 4 i-chunks of 512
NJ = N // P     # 16 j-tiles of 128

_CACHE = {}


def _build_nc(repeat=1):
    if repeat in _CACHE:
        return _CACHE[repeat]
    import concourse.tile as tile
    from concourse import bacc, mybir
    from concourse.masks import make_identity

    f32 = mybir.dt.float32
    bf16 = mybir.dt.bfloat16
    i16 = mybir.dt.int16
    Exp = mybir.ActivationFunctionType.Exp
    # Schraudolph exp approximation constants (bf16 bit pattern via int16):
    # i16 = round(x * 2^7/ln2 + (127*2^7 - C)); bits reinterpreted as bf16.
    SCH_A = float(2 ** 7 / np.log(2))
    SCH_B = float(127 * 2 ** 7 - 486408.0 / 65536.0)
    # exp engine per j-tile in steady loops (first loop runs all-ACT).
    # GPSIMD cannot touch PSUM on TRN2, so only ACT (true exp) and DVE
    # (Schraudolph) share the exp work.
    SCH_DVE_JTS = (1, 3, 5, 7, 9, 11, 14)
    SCH_DVE_JTS_FIRST = (5, 9, 13)

    nc = bacc.Bacc("TRN2", target_bir_lowering=False, debug=False)
    # hd packs [wqkT | x chunk0] so the first-matmul dependency set arrives
    # in one DMA per row-tile (HWDGE serializes at ~625ns/DMA).
    hd_d = nc.dram_tensor("hd", [DIM, 1024], bf16, kind="ExternalInput").ap()
    xc1_d = nc.dram_tensor("xc1", [P, 1024], bf16, kind="ExternalInput").ap()
    wv_d = nc.dram_tensor("wv2", [P, 512], bf16, kind="ExternalInput").ap()
    xr_d = nc.dram_tensor("xrest", [P, 2560], bf16, kind="ExternalInput").ap()
    y_d = nc.dram_tensor("yT", [N, DIM], bf16, kind="ExternalOutput").ap()

    with tile.TileContext(nc) as tc, ExitStack() as ctx:
        persist = ctx.enter_context(tc.tile_pool(name="persist", bufs=1))

        # dependency-free warm-up first: ramp the PE p-state through the
        # DMA-wait window (memset on the otherwise-idle Pool engine)
        warm = persist.tile([P, P], bf16, tag="warm", name="warm")
        nc.gpsimd.memset(warm[:], 0.0)

        # head DMAs: 4 total ([wqkT|x_c0|wvT] and [x_c123|woutT] per
        # row-tile), everything lands directly as bf16, no on-chip casts
        wqk = []
        xsb = {}
        wv = []
        wob = []
        for r in range(2):
            hb = persist.tile([P, 1024], bf16, tag=f"hdb{r}", name=f"hdb{r}")
            nc.sync.dma_start(hb[:], hd_d[r * P:(r + 1) * P, :])
            wqk.append(hb[:, 0:512])
            xsb[(r, 0)] = hb[:, 512:1024]
        xc1 = persist.tile([P, 1024], bf16, tag="xc1", name="xc1")
        nc.sync.dma_start(xc1[:], xc1_d[:, :])
        wvt = persist.tile([P, 512], bf16, tag="wv2", name="wv2")
        nc.sync.dma_start(wvt[:], wv_d[:, :])
        wv.append(wvt[:, 0:256])
        wv.append(wvt[:, 256:512])
        xrest = persist.tile([P, 2560], bf16, tag="xrest", name="xrest")
        nc.sync.dma_start(xrest[:], xr_d[:, :])
        for r in range(2):
            xsb[(r, 1)] = xc1[:, r * 512:(r + 1) * 512]
            for c in range(2, NI):
                xsb[(r, c)] = xrest[:, (c - 2) * 1024 + r * 512:(c - 2) * 1024 + (r + 1) * 512]
            wob.append(xrest[:, 2048 + r * 256:2048 + (r + 1) * 256])

        # identity for the tail's PE transposes (built during the DMA wait)
        ident = persist.tile([P, P], bf16, tag="ident", name="ident")
        make_identity(nc, ident[:])

        # ---- Stage A: minimal upfront QKV; the rest is interleaved into the
        # early attention loops' exp-wait windows.
        qkt = {}
        vT = [None] * NJ

        def emit_qk(pool, m, c, evict=None):
            ps = pool.tile([P, 512], f32, tag=pool._qkv_tag, name="qkps")
            for r in range(2):
                nc.tensor.matmul(
                    ps[:],
                    wqk[r][:, m * P:(m + 1) * P],
                    xsb[(r, c)],
                    start=(r == 0), stop=(r == 1),
                )
            t = persist.tile([P, 512], bf16, tag=f"qk{m}_{c}", name=f"qk{m}_{c}")
            if evict is nc.scalar:
                nc.scalar.copy(t[:], ps[:])
            else:
                (evict or nc.vector).tensor_copy(t[:], ps[:])
            qkt[(m, c)] = t

        def emit_v(pool, j, evict=None):
            ps = pool.tile([P, 256], f32, tag=pool._qkv_tag, name="vps")
            for r in range(2):
                nc.tensor.matmul(
                    ps[:],
                    xsb[(r, j // 4)][:, (j % 4) * P:(j % 4 + 1) * P],
                    wv[r][:],
                    start=(r == 0), stop=(r == 1),
                )
            t = persist.tile([P, 4 * 65], bf16, tag=f"vT{j}", name=f"vT{j}")
            tv = t[:].rearrange("p (h w) -> p h w", h=4)
            nc.gpsimd.memset(tv[:, :, 64:65], 1.0)
            (evict or nc.vector).tensor_copy(tv[:, :, 0:64], ps[:].rearrange("p (h w) -> p h w", h=4))
            vT[j] = t

        with tc.tile_pool(name="qkvps", bufs=2, space="PSUM") as qp:
            qp._qkv_tag = "qkps"
            # dependency-free warm-up matmuls: ramp the PE p-state past the
            # ~3us HAM window before the first real (DMA-gated) matmul lands
            wps = qp.tile([P, 512], f32, tag="qkps", name="warmps")
            for wi in range(26):
                nc.tensor.matmul(wps[:, 0:P], warm[:], warm[:],
                                 start=(wi == 0), stop=(wi == 25))
            emit_qk(qp, 2, 0, evict=nc.scalar)   # k heads 0,1 chunk 0
            emit_qk(qp, 0, 0)                    # q heads 0,1 chunk 0

        # ---- Stage B: attention + projection ----
        on = []
        for p2 in range(2):
            t = persist.tile([P, N], bf16, tag=f"on{p2}", name=f"on{p2}")
            on.append(t)
        small = ctx.enter_context(tc.tile_pool(name="small", bufs=4))
        tinp = ctx.enter_context(tc.tile_pool(name="tinp", bufs=3))
        expool = ctx.enter_context(tc.tile_pool(name="expool", bufs=32))
        simp = ctx.enter_context(tc.tile_pool(name="simp", bufs=3, space="PSUM"))
        avp = ctx.enter_context(tc.tile_pool(name="avp", bufs=1, space="PSUM"))
        yout = ctx.enter_context(tc.tile_pool(name="yout", bufs=3))

        yp = simp  # aux PSUM (deferred QKV, transpose, proj) borrows sim slots
        yp._qkv_tag = "sim"
        # Deferred QKV pieces, interleaved into the early attention loops
        # between exp(jt) and av(jt) where the PE waits on exp anyway.
        # Constraints: v(j) before AV(j) at jt=j+3; k01_cX before jt=4X;
        # k-tiles of a pair before that pair's loop; q_cX before ic=X.
        deferred = {
            (0, 0): {
                0: [("v", 0)],
                1: [("v", 1), ("v", 2)],
                2: [("v", 3), ("qk", 2, 1)],
                3: [("v", 4), ("v", 5)],
                4: [("v", 6), ("qk", 2, 2)],
                5: [("v", 7), ("v", 8)],
                6: [("v", 9), ("qk", 2, 3)],
                7: [("v", 10), ("v", 11)],
                8: [("v", 12), ("qk", 3, 0)],
                9: [("v", 13), ("v", 14)],
                10: [("v", 15), ("qk", 3, 1)],
                11: [("qk", 3, 2)],
                12: [("qk", 3, 3)],
                13: [("qk", 1, 0)],
                14: [("qk", 0, 1)],
            },
            (0, 1): {1: [("qk", 1, 1)]},   # q23_c1, used at (1,1)
            (1, 0): {1: [("qk", 0, 2)]},   # q01_c2, used at (2,0)
            (1, 1): {1: [("qk", 1, 2)]},   # q23_c2, used at (2,1)
            (2, 0): {1: [("qk", 0, 3)]},   # q01_c3, used at (3,0)
            (2, 1): {1: [("qk", 1, 3)]},   # q23_c3, used at (3,1)
        }

        pend_ydma = []
        ysb_cur = [None]

        def emit_proj(ic, s, act_evict=False):
            i0 = ic * 512 + s * P
            pyp = yp.tile([P, 256], f32, tag="sim", name="yproj")
            for ct in range(2):
                nc.tensor.matmul(pyp[:], on[ct][:, i0:i0 + P], wob[ct][:],
                                 start=(ct == 0), stop=(ct == 1))
            if s == 0:
                ysb_cur[0] = yout.tile([P, 1024], bf16, tag="ysb", name="ysb")
            ysb = ysb_cur[0]
            with tc.high_priority():
                # jump the engine queue: the sim-ring slot pyp borrows stays
                # blocked until this evict runs
                if act_evict:
                    nc.scalar.copy(ysb[:, s * 256:(s + 1) * 256], pyp[:])
                else:
                    nc.vector.tensor_copy(ysb[:, s * 256:(s + 1) * 256], pyp[:])
            if s == 3:
                # one DMA for the whole 512-row chunk: y rows i = s*128 + p
                pend_ydma.append((ic, ysb))

        def emit_ydma():
            ic, ysb = pend_ydma.pop(0)
            nc.sync.dma_start(
                y_d[ic * 512:(ic + 1) * 512, :].rearrange("(s p) o -> p s o", s=4),
                ysb[:].rearrange("p (s o) -> p s o", s=4))

        def emit_av(entry):
            # transposed AV: av[i, d|Z] += ex_slice^T @ vT_head (65 cycles).
            # PSUM lazy-zeroing is per 2KB bank: only the first matmul of a
            # bank may set start, only the last sets stop.
            ex, jt2, av, pp = entry
            for h2 in range(2):
                hh = 2 * pp + h2
                for s in range(4):
                    off = h2 * 512 + s * 128
                    nc.tensor.matmul(
                        av[:, off:off + 65],
                        ex[:, off:off + 128],
                        vT[jt2][:, hh * 65:hh * 65 + 65],
                        start=(jt2 == 0 and s == 0),
                        stop=(jt2 == NJ - 1 and s == 3),
                    )

        def emit_norm(entry, split4=False):
            # invZ for all 8 slots in one strided reciprocal, then
            # broadcast-multiplies into the packed [i, c] tile.
            pp, icc, av = entry
            avv = av[:].rearrange("q (h s c) -> q h s c", h=2, s=4)
            iz = small.tile([P, 8], f32, tag="iz", name="iz")
            nc.vector.reciprocal(
                iz[:].rearrange("q (h s c) -> q h s c", h=2, s=4),
                avv[:, :, :, 64:65])
            # one packed [i, 4s x (2h x 64c)] tile; 2 broadcast-multiply ops
            # (one per h2) replace 8 per-slot scaled copies
            ta = tinp.tile([P, 512], bf16, tag="tins", name="tins")
            tav = ta[:].rearrange("q (s t c) -> q s t c", s=4, t=2)
            # dim order (s, h2, c) on BOTH sides -> a single strided
            # TensorTensor normalizes all 8 slots (no inter-op queue gap
            # on the loop-boundary critical path)
            avp_ = av[:].rearrange("q (h s c) -> q s h c", h=2, s=4)
            izp = iz[:].rearrange("q (h s) -> q s h ()", h=2)
            if not split4:
                nc.vector.tensor_tensor(
                    tav[:, :, :, :],
                    avp_[:, :, :, 0:64],
                    izp.broadcast_to([P, 4, 2, 64]),
                    mybir.AluOpType.mult)
            else:
                # tail: two s-pair pieces so the first transposes start early
                for s0, s1 in ((0, 2), (2, 4)):
                    ns = s1 - s0
                    nc.vector.tensor_tensor(
                        tav[:, s0:s1, :, :],
                        avp_[:, s0:s1, :, 0:64],
                        izp[:, s0:s1].broadcast_to([P, ns, 2, 64]),
                        mybir.AluOpType.mult)
            return [(pp, icc, s, ta) for s in range(4)]

        def emit_trans(job):
            # [i, c] -> [c, i] via the DMA XBAR (SBUF->SBUF, bf16): zero
            # engine cost, only HWDGE ring occupancy.
            pp, icc, s, ta = job
            nc.sync.dma_start_transpose(
                out=on[pp][:, icc * 512 + s * P: icc * 512 + (s + 1) * P],
                in_=ta[:, s * P:(s + 1) * P])

        # Software pipelines persisting ACROSS (ic, p) loops: av matmuls run
        # 3 jts behind their exp; the norm of loop L runs at loop L+1 jt=3
        # (just after L's last AV, emitted at L+1 jt=2, and before L+1's
        # AV(0) reclaims the single av PSUM buffer); transposes at jt=4/5.
        pend_av = []
        pend_norm = []
        pend_trans = []
        pending_proj = None
        for rep in range(repeat):
            for ic in range(NI):
                for p in range(2):
                    qt = qkt[(p, ic)]
                    av = avp.tile([P, 1024], f32, tag="av", name="av")
                    dmap = dict(deferred.get((ic, p), {})) if rep == 0 else {}
                    if p == 0 and pending_proj is not None:
                        for s in range(4):
                            dmap.setdefault(12 + s, []).append(("proj", pending_proj, s))
                        pending_proj = None
                    for jt in range(NJ):
                        sim = simp.tile([P, 1024], f32, tag="sim", name="sim")
                        kt = qkt[(2 + p, jt // 4)]
                        ko = (jt % 4) * P
                        nc.tensor.matmul(sim[:, 0:512], kt[0:64, ko:ko + P],
                                         qt[0:64, :], start=True, stop=True)
                        nc.tensor.matmul(sim[:, 512:1024], kt[64:128, ko:ko + P],
                                         qt[64:128, :], start=True, stop=True)
                        first_loop = (ic, p) == (0, 0) and rep == 0
                        sch_set = SCH_DVE_JTS_FIRST if first_loop else SCH_DVE_JTS
                        if jt in sch_set:
                            exi = expool.tile([P, 1024], i16, tag="ex", name="exi")
                            nc.vector.tensor_scalar(
                                exi[:], sim[:], SCH_A, SCH_B,
                                mybir.AluOpType.mult, mybir.AluOpType.add)
                            ex = exi[:].bitcast(bf16)
                        else:  # ACT true exp
                            exb = expool.tile([P, 1024], bf16, tag="ex", name="ex")
                            nc.scalar.activation(exb[:], sim[:], Exp)
                            ex = exb[:]
                        if jt == 0 and pend_ydma:
                            emit_ydma()
                        if jt in (7, 8) and pend_trans:
                            emit_trans(pend_trans.pop(0))
                            emit_trans(pend_trans.pop(0))
                        for piece in dmap.get(jt, []):
                            if piece[0] == "v":
                                emit_v(yp, piece[1])
                            elif piece[0] == "qk":
                                emit_qk(yp, piece[1], piece[2])
                            else:
                                emit_proj(piece[1], piece[2], act_evict=True)
                        pend_av.append((ex, jt, av, p))
                        if len(pend_av) > 4:
                            emit_av(pend_av.pop(0))
                    # drain the AV pipeline and normalize NOW (ahead of the
                    # next loop's exps in priority order) so the single av
                    # PSUM buffer frees early at the loop boundary.
                    while pend_av:
                        emit_av(pend_av.pop(0))
                    pend_trans.extend(emit_norm((p, ic, av),
                                                 split4=(ic == NI - 1 and p == 1)))
                    if p == 1:
                        pending_proj = ic
        # tail: low-latency PE transposes (borrowing sim PSUM slots), the
        # final projections interleaved per i-subtile, trans-evicts on ACT
        # (parallel to the DVE ysb evicts), and per-subtile output DMAs so
        # the last DMA is small and earlier ones overlap the tail compute
        for s in range(4):
            pp, icc, s_, ta = pend_trans.pop(0)
            tp = yp.tile([P, P], bf16, tag="sim", name="tps")
            nc.tensor.transpose(tp[:], ta[:, s_ * P:(s_ + 1) * P], ident[:])
            nc.vector.tensor_copy(
                on[pp][:, icc * 512 + s_ * P: icc * 512 + (s_ + 1) * P], tp[:])
            emit_proj(pending_proj, s_, act_evict=True)
        while pend_ydma:
            pend_ydma.pop(0)
        ic3, ysb = pending_proj, ysb_cur[0]
        nc.sync.dma_start(
            y_d[ic3 * 512:ic3 * 512 + 256, :].rearrange("(s p) o -> p s o", s=2),
            ysb[:, 0:512].rearrange("p (s o) -> p s o", s=2))
        for s_ in (2, 3):
            r0 = ic3 * 512 + s_ * P
            nc.sync.dma_start(y_d[r0:r0 + P, :], ysb[:, s_ * 256:(s_ + 1) * 256])

    nc.compile()
    _CACHE[repeat] = nc
    return nc


def _shard_inputs(x, w_qkv, w_out):
    import ml_dtypes
    bf16 = ml_dtypes.bfloat16
    in_maps = []
    for c in range(8):
        b, g = c // 2, c % 2
        wq = w_qkv[g * 256:(g + 1) * 256] * SCALE
        wk = w_qkv[512 + g * 256:512 + (g + 1) * 256]
        wvv = w_qkv[1024 + g * 256:1024 + (g + 1) * 256]
        wqkT = np.concatenate([wq, wk], 0).T
        hd = np.concatenate([wqkT, x[b][:, 0:512]], axis=1)
        xb = x[b]
        woT = w_out[:, g * 256:(g + 1) * 256].T
        xc1 = np.concatenate([xb[0:128, 512:1024], xb[128:256, 512:1024]], axis=1)
        xrest = np.concatenate(
            [xb[0:128, 1024:1536], xb[128:256, 1024:1536],
             xb[0:128, 1536:2048], xb[128:256, 1536:2048],
             woT[0:128], woT[128:256]], axis=1)
        wvT = wvv.T
        in_maps.append({
            "hd": np.ascontiguousarray(hd.astype(bf16)),
            "xc1": np.ascontiguousarray(xc1.astype(bf16)),
            "wv2": np.ascontiguousarray(
                np.concatenate([wvT[0:128], wvT[128:256]], axis=1).astype(bf16)),
            "xrest": np.ascontiguousarray(xrest.astype(bf16)),
        })
    return in_maps


def kernel(x, w_qkv, w_out, b_out):
    from concourse.bass_utils import run_bass_kernel_spmd
    x = np.asarray(x, dtype=np.float32)
    w_qkv = np.asarray(w_qkv, dtype=np.float32)
    w_out = np.asarray(w_out, dtype=np.float32)
    b_out = np.asarray(b_out, dtype=np.float32)

    nc = _build_nc()
    in_maps = _shard_inputs(x, w_qkv, w_out)
    res = run_bass_kernel_spmd(nc, in_maps, core_ids=list(range(8)))
    y = np.empty((B, DIM, N), np.float32)
    for b in range(B):
        yT = (np.asarray(res.results[2 * b]["yT"], dtype=np.float32)
              + np.asarray(res.results[2 * b + 1]["yT"], dtype=np.float32))
        y[b] = yT.T + b_out[:, None]
    return y


# revision 33
# speedup vs baseline: 1.0069x; 1.0019x over previous
"""Multi-head attention (b=4, h=8, d=64, n=2048, dim=256) on 8 TRN2 NeuronCores.

Sharding: core c handles batch b=c//2 and head-group g=c%2 (4 heads).
Each core computes its 4 heads' attention plus the partial output
projection y_part = w_out[:, g*256:(g+1)*256] @ attn_out, returned
transposed as yT [n, 256].  Host: y[b] = (yT[2b] + yT[2b+1]).T + b_out.
No cross-core collectives (the hint's b*h split, done host-side).
All inputs are pre-cast to bf16 on the host during sharding.

Per-core pipeline (n=2048, 4 heads as 2 head-pairs, matmul operands bf16,
PSUM accumulation fp32).  The AV stage exploits the PE cost law
(cycles = moving/output free size, stationary loads are free):
  QKV:    q,k via w-stationary matmuls; v^T via x-stationary matmuls
          (v lands in the [j, d] layout AV needs).  v^T stored with a
          ones column per head ([128, 4*65]) so AV also computes Z free.
  scores: sim_T[j, i] = k^T q, two heads of a pair in one [128, 1024]
          PSUM supertile (512 i-cols each).
  exp:    one instruction per supertile -> bf16 SBUF.  Max-subtraction
          skipped (|sim| <~ 8).  Tiles are split across THREE engines:
          ACT runs true Exp; DVE and GPSIMD(Pool) run a one-instruction
          Schraudolph exp (int16 bit-trick -> bf16).  Scattered approx
          tiles dilute the ~1.7% elementwise error through the softmax
          denominator.
  AV:     TRANSPOSED layout: av[i, d|Z] += ex_tile^T @ vT with the
          128-col ex slice stationary and vT [128, 65] moving -> 65
          cycles per matmul instead of 512 (8 small matmuls per
          supertile, PSUM-accumulated over the 16 j-tiles; i on
          partitions).  This halves+ the AV PE time vs the classic
          orientation.
  norm:   Z sits in PSUM col 64 of each av slot; one strided DVE
          reciprocal -> invZ [128, 8] SBUF, then 8 per-partition-scaled
          copies (GPSIMD tensor_scalar ptr) write normalized bf16 into
          packed [128 i, 128 c] tiles.
  trans:  PE transpose (identity matmul, 128 cycles) flips each packed
          tile to [c, i] into PSUM-bf16; DVE evicts (2x mode) into the
          persistent out_cloc [128 c, 2048 i] per pair.
  proj:   yT[i, o] with out_cloc stationary, w_outT moving; f32 evict,
          DMA per i-tile.

Scheduling: QKV is deferred piecewise into the first loops' exp-wait
windows; AV runs a 3-iteration software pipeline behind exp and persists
across loop boundaries; norm of loop L runs at loop L+1 jt=3 (before
L+1's first AV write reuses the single av PSUM buffer), transposes at
jt=4/5, projection of chunk ic at the next p==0 loop jts 12..15.
PSUM: 3x2-bank sim supertiles (also borrowed by deferred QKV, transpose
and projection pieces) + 2 av banks = 8 banks exactly.
"""

import numpy as np
from contextlib import ExitStack

DIM = 256
HEADS = 8
DH = 64
HID = 512
N = 2048
B = 4
SCALE = DH ** -0.5
P = 128
NI = N // 512   # 4 i-chunks of 512
NJ = N // P     # 16 j-tiles of 128

_CACHE = {}


def _build_nc(repeat=1):
    if repeat in _CACHE:
        return _CACHE[repeat]
    import concourse.tile as tile
    from concourse import bacc, mybir
    from concourse.masks import make_identity

    f32 = mybir.dt.float32
    bf16 = mybir.dt.bfloat16
    i16 = mybir.dt.int16
    Exp = mybir.ActivationFunctionType.Exp
    # Schraudolph exp approximation constants (bf16 bit pattern via int16):
    # i16 = round(x * 2^7/ln2 + (127*2^7 - C)); bits reinterpreted as bf16.
    SCH_A = float(2 ** 7 / np.log(2))
    SCH_B = float(127 * 2 ** 7 - 486408.0 / 65536.0)
    # exp engine per j-tile in steady loops (first loop runs all-ACT).
    # GPSIMD cannot touch PSUM on TRN2, so only ACT (true exp) and DVE
    # (Schraudolph) share the exp work.
    SCH_DVE_JTS = (1, 3, 5, 7, 9, 11, 14)
    SCH_DVE_JTS_FIRST = (5, 9, 13)

    nc = bacc.Bacc("TRN2", target_bir_lowering=False, debug=False)
    # hd packs [wqkT | x chunk0] so the first-matmul dependency set arrives
    # in one DMA per row-tile (HWDGE serializes at ~625ns/DMA).
    hd_d = nc.dram_tensor("hd", [DIM, 1024], bf16, kind="ExternalInput").ap()
    xc1_d = nc.dram_tensor("xc1", [P, 1024], bf16, kind="ExternalInput").ap()
    wv_d = nc.dram_tensor("wv2", [P, 512], bf16, kind="ExternalInput").ap()
    xr_d = nc.dram_tensor("xrest", [P, 2560], bf16, kind="ExternalInput").ap()
    y_d = nc.dram_tensor("yT", [N, DIM], bf16, kind="ExternalOutput").ap()

    with tile.TileContext(nc) as tc, ExitStack() as ctx:
        persist = ctx.enter_context(tc.tile_pool(name="persist", bufs=1))

        # dependency-free warm-up first: ramp the PE p-state through the
        # DMA-wait window (memset on the otherwise-idle Pool engine)
        warm = persist.tile([P, P], bf16, tag="warm", name="warm")
        nc.gpsimd.memset(warm[:], 0.0)

        # head DMAs: 4 total ([wqkT|x_c0|wvT] and [x_c123|woutT] per
        # row-tile), everything lands directly as bf16, no on-chip casts
        wqk = []
        xsb = {}
        wv = []
        wob = []
        for r in range(2):
            hb = persist.tile([P, 1024], bf16, tag=f"hdb{r}", name=f"hdb{r}")
            nc.sync.dma_start(hb[:], hd_d[r * P:(r + 1) * P, :])
            wqk.append(hb[:, 0:512])
            xsb[(r, 0)] = hb[:, 512:1024]
        xc1 = persist.tile([P, 1024], bf16, tag="xc1", name="xc1")
        nc.sync.dma_start(xc1[:], xc1_d[:, :])
        wvt = persist.tile([P, 512], bf16, tag="wv2", name="wv2")
        nc.sync.dma_start(wvt[:], wv_d[:, :])
        wv.append(wvt[:, 0:256])
        wv.append(wvt[:, 256:512])
        xrest = persist.tile([P, 2560], bf16, tag="xrest", name="xrest")
        nc.sync.dma_start(xrest[:], xr_d[:, :])
        for r in range(2):
            xsb[(r, 1)] = xc1[:, r * 512:(r + 1) * 512]
            for c in range(2, NI):
                xsb[(r, c)] = xrest[:, (c - 2) * 1024 + r * 512:(c - 2) * 1024 + (r + 1) * 512]
            wob.append(xrest[:, 2048 + r * 256:2048 + (r + 1) * 256])

        # identity for the tail's PE transposes (built during the DMA wait)
        ident = persist.tile([P, P], bf16, tag="ident", name="ident")
        make_identity(nc, ident[:])

        # ---- Stage A: minimal upfront QKV; the rest is interleaved into the
        # early attention loops' exp-wait windows.
        qkt = {}
        vT = [None] * NJ

        def emit_qk(pool, m, c, evict=None):
            ps = pool.tile([P, 512], f32, tag=pool._qkv_tag, name="qkps")
            for r in range(2):
                nc.tensor.matmul(
                    ps[:],
                    wqk[r][:, m * P:(m + 1) * P],
                    xsb[(r, c)],
                    start=(r == 0), stop=(r == 1),
                )
            t = persist.tile([P, 512], bf16, tag=f"qk{m}_{c}", name=f"qk{m}_{c}")
            if evict is nc.scalar:
                nc.scalar.copy(t[:], ps[:])
            else:
                (evict or nc.vector).tensor_copy(t[:], ps[:])
            qkt[(m, c)] = t

        def emit_v(pool, j, evict=None):
            ps = pool.tile([P, 256], f32, tag=pool._qkv_tag, name="vps")
            for r in range(2):
                nc.tensor.matmul(
                    ps[:],
                    xsb[(r, j // 4)][:, (j % 4) * P:(j % 4 + 1) * P],
                    wv[r][:],
                    start=(r == 0), stop=(r == 1),
                )
            t = persist.tile([P, 4 * 65], bf16, tag=f"vT{j}", name=f"vT{j}")
            tv = t[:].rearrange("p (h w) -> p h w", h=4)
            nc.gpsimd.memset(tv[:, :, 64:65], 1.0)
            (evict or nc.vector).tensor_copy(tv[:, :, 0:64], ps[:].rearrange("p (h w) -> p h w", h=4))
            vT[j] = t

        with tc.tile_pool(name="qkvps", bufs=2, space="PSUM") as qp:
            qp._qkv_tag = "qkps"
            # dependency-free warm-up matmuls: ramp the PE p-state past the
            # ~3us HAM window before the first real (DMA-gated) matmul lands
            wps = qp.tile([P, 512], f32, tag="qkps", name="warmps")
            for wi in range(26):
                nc.tensor.matmul(wps[:, 0:P], warm[:], warm[:],
                                 start=(wi == 0), stop=(wi == 25))
            emit_qk(qp, 2, 0, evict=nc.scalar)   # k heads 0,1 chunk 0
            emit_qk(qp, 0, 0)                    # q heads 0,1 chunk 0

        # ---- Stage B: attention + projection ----
        on = []
        for p2 in range(2):
            t = persist.tile([P, N], bf16, tag=f"on{p2}", name=f"on{p2}")
            on.append(t)
        small = ctx.enter_context(tc.tile_pool(name="small", bufs=4))
        tinp = ctx.enter_context(tc.tile_pool(name="tinp", bufs=3))
        expool = ctx.enter_context(tc.tile_pool(name="expool", bufs=24))
        simp = ctx.enter_context(tc.tile_pool(name="simp", bufs=3, space="PSUM"))
        avp = ctx.enter_context(tc.tile_pool(name="avp", bufs=1, space="PSUM"))
        yout = ctx.enter_context(tc.tile_pool(name="yout", bufs=3))

        yp = simp  # aux PSUM (deferred QKV, transpose, proj) borrows sim slots
        yp._qkv_tag = "sim"
        # Deferred QKV pieces, interleaved into the early attention loops
        # between exp(jt) and av(jt) where the PE waits on exp anyway.
        # Constraints: v(j) before AV(j) at jt=j+3; k01_cX before jt=4X;
        # k-tiles of a pair before that pair's loop; q_cX before ic=X.
        deferred = {
            (0, 0): {
                0: [("v", 0)],
                1: [("v", 1), ("v", 2)],
                2: [("v", 3), ("qk", 2, 1)],
                3: [("v", 4), ("v", 5)],
                4: [("v", 6), ("qk", 2, 2)],
                5: [("v", 7), ("v", 8)],
                6: [("v", 9), ("qk", 2, 3)],
                7: [("v", 10), ("v", 11)],
                8: [("v", 12), ("qk", 3, 0)],
                9: [("v", 13), ("v", 14)],
                10: [("v", 15), ("qk", 3, 1)],
                11: [("qk", 3, 2)],
                12: [("qk", 3, 3)],
                13: [("qk", 1, 0)],
                14: [("qk", 0, 1)],
            },
            (0, 1): {1: [("qk", 1, 1)]},   # q23_c1, used at (1,1)
            (1, 0): {1: [("qk", 0, 2)]},   # q01_c2, used at (2,0)
            (1, 1): {1: [("qk", 1, 2)]},   # q23_c2, used at (2,1)
            (2, 0): {1: [("qk", 0, 3)]},   # q01_c3, used at (3,0)
            (2, 1): {1: [("qk", 1, 3)]},   # q23_c3, used at (3,1)
        }

        pend_ydma = []
        ysb_cur = [None]

        def emit_proj(ic, s, act_evict=False):
            i0 = ic * 512 + s * P
            pyp = yp.tile([P, 256], f32, tag="sim", name="yproj")
            for ct in range(2):
                nc.tensor.matmul(pyp[:], on[ct][:, i0:i0 + P], wob[ct][:],
                                 start=(ct == 0), stop=(ct == 1))
            if s == 0:
                ysb_cur[0] = yout.tile([P, 1024], bf16, tag="ysb", name="ysb")
            ysb = ysb_cur[0]
            with tc.high_priority():
                # jump the engine queue: the sim-ring slot pyp borrows stays
                # blocked until this evict runs
                if act_evict:
                    nc.scalar.copy(ysb[:, s * 256:(s + 1) * 256], pyp[:])
                else:
                    nc.vector.tensor_copy(ysb[:, s * 256:(s + 1) * 256], pyp[:])
            if s == 3:
                # one DMA for the whole 512-row chunk: y rows i = s*128 + p
                pend_ydma.append((ic, ysb))

        def emit_ydma():
            ic, ysb = pend_ydma.pop(0)
            nc.sync.dma_start(
                y_d[ic * 512:(ic + 1) * 512, :].rearrange("(s p) o -> p s o", s=4),
                ysb[:].rearrange("p (s o) -> p s o", s=4))

        def emit_av(entry):
            # transposed AV: av[i, d|Z] += ex_slice^T @ vT_head (65 cycles).
            # PSUM lazy-zeroing is per 2KB bank: only the first matmul of a
            # bank may set start, only the last sets stop.
            ex, jt2, av, pp = entry
            for h2 in range(2):
                hh = 2 * pp + h2
                for s in range(4):
                    off = h2 * 512 + s * 128
                    nc.tensor.matmul(
                        av[:, off:off + 65],
                        ex[:, off:off + 128],
                        vT[jt2][:, hh * 65:hh * 65 + 65],
                        start=(jt2 == 0 and s == 0),
                        stop=(jt2 == NJ - 1 and s == 3),
                    )

        def emit_norm(entry, split4=False):
            # invZ for all 8 slots in one strided reciprocal, then
            # broadcast-multiplies into the packed [i, c] tile.
            pp, icc, av = entry
            avv = av[:].rearrange("q (h s c) -> q h s c", h=2, s=4)
            iz = small.tile([P, 8], f32, tag="iz", name="iz")
            nc.vector.reciprocal(
                iz[:].rearrange("q (h s c) -> q h s c", h=2, s=4),
                avv[:, :, :, 64:65])
            # one packed [i, 4s x (2h x 64c)] tile; 2 broadcast-multiply ops
            # (one per h2) replace 8 per-slot scaled copies
            ta = tinp.tile([P, 512], bf16, tag="tins", name="tins")
            tav = ta[:].rearrange("q (s t c) -> q s t c", s=4, t=2)
            # dim order (s, h2, c) on BOTH sides -> a single strided
            # TensorTensor normalizes all 8 slots (no inter-op queue gap
            # on the loop-boundary critical path)
            avp_ = av[:].rearrange("q (h s c) -> q s h c", h=2, s=4)
            izp = iz[:].rearrange("q (h s) -> q s h ()", h=2)
            if not split4:
                nc.vector.tensor_tensor(
                    tav[:, :, :, :],
                    avp_[:, :, :, 0:64],
                    izp.broadcast_to([P, 4, 2, 64]),
                    mybir.AluOpType.mult)
            else:
                # tail: two s-pair pieces so the first transposes start early
                for s0, s1 in ((0, 2), (2, 4)):
                    ns = s1 - s0
                    nc.vector.tensor_tensor(
                        tav[:, s0:s1, :, :],
                        avp_[:, s0:s1, :, 0:64],
                        izp[:, s0:s1].broadcast_to([P, ns, 2, 64]),
                        mybir.AluOpType.mult)
            return [(pp, icc, s, ta) for s in range(4)]

        def emit_trans(job):
            # [i, c] -> [c, i] via the DMA XBAR (SBUF->SBUF, bf16): zero
            # engine cost, only HWDGE ring occupancy.
            pp, icc, s, ta = job
            nc.sync.dma_start_transpose(
                out=on[pp][:, icc * 512 + s * P: icc * 512 + (s + 1) * P],
                in_=ta[:, s * P:(s + 1) * P])

        # Software pipelines persisting ACROSS (ic, p) loops: av matmuls run
        # 3 jts behind their exp; the norm of loop L runs at loop L+1 jt=3
        # (just after L's last AV, emitted at L+1 jt=2, and before L+1's
        # AV(0) reclaims the single av PSUM buffer); transposes at jt=4/5.
        pend_av = []
        pend_norm = []
        pend_trans = []
        pending_proj = None
        for rep in range(repeat):
            for ic in range(NI):
                for p in range(2):
                    qt = qkt[(p, ic)]
                    av = avp.tile([P, 1024], f32, tag="av", name="av")
                    dmap = dict(deferred.get((ic, p), {})) if rep == 0 else {}
                    if p == 0 and pending_proj is not None:
                        for s in range(4):
                            dmap.setdefault(12 + s, []).append(("proj", pending_proj, s))
                        pending_proj = None
                    for jt in range(NJ):
                        sim = simp.tile([P, 1024], f32, tag="sim", name="sim")
                        kt = qkt[(2 + p, jt // 4)]
                        ko = (jt % 4) * P
                        nc.tensor.matmul(sim[:, 0:512], kt[0:64, ko:ko + P],
                                         qt[0:64, :], start=True, stop=True)
                        nc.tensor.matmul(sim[:, 512:1024], kt[64:128, ko:ko + P],
                                         qt[64:128, :], start=True, stop=True)
                        first_loop = (ic, p) == (0, 0) and rep == 0
                        sch_set = SCH_DVE_JTS_FIRST if first_loop else SCH_DVE_JTS
                        if jt in sch_set:
                            exi = expool.tile([P, 1024], i16, tag="ex", name="exi")
                            nc.vector.tensor_scalar(
                                exi[:], sim[:], SCH_A, SCH_B,
                                mybir.AluOpType.mult, mybir.AluOpType.add)
                            ex = exi[:].bitcast(bf16)
                        else:  # ACT true exp
                            exb = expool.tile([P, 1024], bf16, tag="ex", name="ex")
                            nc.scalar.activation(exb[:], sim[:], Exp)
                            ex = exb[:]
                        if jt == 0 and pend_ydma:
                            emit_ydma()
                        if jt in (7, 8) and pend_trans:
                            emit_trans(pend_trans.pop(0))
                            emit_trans(pend_trans.pop(0))
                        for piece in dmap.get(jt, []):
                            if piece[0] == "v":
                                emit_v(yp, piece[1])
                            elif piece[0] == "qk":
                                emit_qk(yp, piece[1], piece[2])
                            else:
                                emit_proj(piece[1], piece[2], act_evict=True)
                        pend_av.append((ex, jt, av, p))
                        if len(pend_av) > 4:
                            emit_av(pend_av.pop(0))
                    # drain the AV pipeline and normalize NOW (ahead of the
                    # next loop's exps in priority order) so the single av
                    # PSUM buffer frees early at the loop boundary.
                    while pend_av:
                        emit_av(pend_av.pop(0))
                    pend_trans.extend(emit_norm((p, ic, av),
                                                 split4=(ic == NI - 1 and p == 1)))
                    if p == 1:
                        pending_proj = ic
        # tail: low-latency PE transposes (borrowing sim PSUM slots), the
        # final projections interleaved per i-subtile, trans-evicts on ACT
        # (parallel to the DVE ysb evicts), and per-subtile output DMAs so
        # the last DMA is small and earlier ones overlap the tail compute
        for s in range(4):
            pp, icc, s_, ta = pend_trans.pop(0)
            tp = yp.tile([P, P], bf16, tag="sim", name="tps")
            nc.tensor.transpose(tp[:], ta[:, s_ * P:(s_ + 1) * P], ident[:])
            nc.vector.tensor_copy(
                on[pp][:, icc * 512 + s_ * P: icc * 512 + (s_ + 1) * P], tp[:])
            emit_proj(pending_proj, s_, act_evict=True)
        while pend_ydma:
            pend_ydma.pop(0)
        ic3, ysb = pending_proj, ysb_cur[0]
        nc.sync.dma_start(
            y_d[ic3 * 512:ic3 * 512 + 256, :].rearrange("(s p) o -> p s o", s=2),
            ysb[:, 0:512].rearrange("p (s o) -> p s o", s=2))
        for s_ in (2, 3):
            r0 = ic3 * 512 + s_ * P
            nc.sync.dma_start(y_d[r0:r0 + P, :], ysb[:, s_ * 256:(s_ + 1) * 256])

    nc.compile()
    _CACHE[repeat] = nc
    return nc


def _shard_inputs(x, w_qkv, w_out):
    import ml_dtypes
    bf16 = ml_dtypes.bfloat16
    in_maps = []
    for c in range(8):
        b, g = c // 2, c % 2
        wq = w_qkv[g * 256:(g + 1) * 256] * SCALE
        wk = w_qkv[512 + g * 256:512 + (g + 1) * 256]
        wvv = w_qkv[1024 + g * 256:1024 + (g + 1) * 256]
        wqkT = np.concatenate([wq, wk], 0).T
        hd = np.concatenate([wqkT, x[b][:, 0:512]], axis=1)
        xb = x[b]
        woT = w_out[:, g * 256:(g + 1) * 256].T
        xc1 = np.concatenate([xb[0:128, 512:1024], xb[128:256, 512:1024]], axis=1)
        xrest = np.concatenate(
            [xb[0:128, 1024:1536], xb[128:256, 1024:1536],
             xb[0:128, 1536:2048], xb[128:256, 1536:2048],
             woT[0:128], woT[128:256]], axis=1)
        wvT = wvv.T
        in_maps.append({
            "hd": np.ascontiguousarray(hd.astype(bf16)),
            "xc1": np.ascontiguousarray(xc1.astype(bf16)),
            "wv2": np.ascontiguousarray(
                np.concatenate([wvT[0:128], wvT[128:256]], axis=1).astype(bf16)),
            "xrest": np.ascontiguousarray(xrest.astype(bf16)),
        })
    return in_maps


def kernel(x, w_qkv, w_out, b_out):
    from concourse.bass_utils import run_bass_kernel_spmd
    x = np.asarray(x, dtype=np.float32)
    w_qkv = np.asarray(w_qkv, dtype=np.float32)
    w_out = np.asarray(w_out, dtype=np.float32)
    b_out = np.asarray(b_out, dtype=np.float32)

    nc = _build_nc()
    in_maps = _shard_inputs(x, w_qkv, w_out)
    res = run_bass_kernel_spmd(nc, in_maps, core_ids=list(range(8)))
    y = np.empty((B, DIM, N), np.float32)
    for b in range(B):
        yT = (np.asarray(res.results[2 * b]["yT"], dtype=np.float32)
              + np.asarray(res.results[2 * b + 1]["yT"], dtype=np.float32))
        y[b] = yT.T + b_out[:, None]
    return y
